# revision 43
# baseline (speedup 1.0000x reference)
"""CoNystromAttention Trainium2 kernel.

Shard: 8 cores = 4 batches x 2 head-groups (8 heads each). Per core:
one batch b, 8 heads organized as 4 "pairs" (2 heads = 128 partitions).

Math (reference, with Q=K=V=QKV):
  QKV = X[b].T @ Wq[h].T + bq[h]                       [n=4096, d=64]
  Qt  = window-mean(QKV, 64)                           [m=64, d]
  S   = exp(QKV @ Qt.T / 8)     (Beta; Delta = S.T)    [n, m]
  G   = exp(Qt @ Qt.T / 8)
  GD  = G / rowsum(G);  V6 = newton_schulz(GD, 6)      (pinv)
  out = diag(1/r) S V6 diag(1/c) S.T QKV,  r=rowsum(S), c=colsum(S)

Projection via fp8e4 DoubleRow matmuls on host-prepped hi/lo splits of
8*X and 32*W (QKV = (Whi^T(Xhi+Xlo) + Wlo^T Xhi)/256, ~0.1% accurate);
everything downstream (S, transposes, M, NS, final) in bf16; output
DMA'd as bf16 and upcast on the host.  The NS operand K is
error-compensated as gd_hi + gd_lo (two accumulating bf16 matmuls) so
the iteration inverts GD at ~f32 precision.  NS init scale uses the
per-core max (8 heads) instead of the reference's global max; since GD
is row-normalized the rowsum term is exactly 1, so scale=1/max colsum.
"""

import numpy as np

P = 128
N_TOK = 4096
EMBED = 1024
NPAIR = 4            # head-pairs per core (8 heads)
ECH = EMBED // P     # 8 contraction chunks
XCH = 512            # projection chunk (tokens)
NCHP = N_TOK // XCH  # 8 projection chunks
NCH8 = N_TOK // 512  # 8 ST chunks of 512
TCH = N_TOK // P     # 32 token chunks of 128
NS_ITERS = 6

_CACHE = {}


def _build(**_ignored):
    import concourse.mybir as mybir
    from concourse import bacc, bass_isa
    from concourse.tile import TileContext
    from concourse.masks import make_identity

    f32 = mybir.dt.float32
    f32r = mybir.dt.float32r
    bf16 = mybir.dt.bfloat16
    ALU = mybir.AluOpType
    ACTF = mybir.ActivationFunctionType
    AX = mybir.AxisListType

    u8 = mybir.dt.uint8
    bf16_out = mybir.dt.bfloat16
    fp8 = mybir.dt.float8e4

    nc = bacc.Bacc("TRN2", target_bir_lowering=False, debug=False)
    # projection inputs as e4m3 hi/lo splits of 8*X and 32*W (host-prepped);
    # QKV = (Whi^T(Xhi+Xlo) + Wlo^T Xhi)/256 via DoubleRow fp8 matmuls
    Xhi = nc.dram_tensor("Xhi", [EMBED, N_TOK], u8, kind="ExternalInput")
    Xlo = nc.dram_tensor("Xlo", [EMBED, N_TOK], u8, kind="ExternalInput")
    Whi = nc.dram_tensor("Whi", [EMBED, 512], u8, kind="ExternalInput")
    Wlo = nc.dram_tensor("Wlo", [EMBED, 512], u8, kind="ExternalInput")
    bias = nc.dram_tensor("bias", [512], f32, kind="ExternalInput")
    out_d = nc.dram_tensor("out", [N_TOK, 512], bf16_out, kind="ExternalOutput")
    rinv_d = nc.dram_tensor("rinv", [P, TCH, NPAIR, 2], f32, kind="ExternalOutput")

    with TileContext(nc) as tc, (
        tc.tile_pool(name="big", bufs=1)
    ) as big, tc.tile_pool(name="pers", bufs=1) as pers, tc.tile_pool(
        name="nsv", bufs=2
    ) as nsp:
        # ---------------- persistent tiles ----------------
        ident32 = pers.tile([P, P], f32, tag="ident32")
        make_identity(nc, ident32[:])
        identb = pers.tile([P, P], bf16, tag="identb")
        nc.vector.tensor_copy(identb[:], ident32[:])
        iw = []
        for v in (7.0, 15.0, 13.0):
            t = pers.tile([P, NPAIR, P], f32, tag=f"i{int(v)}")
            nc.vector.tensor_scalar_mul(
                t[:],
                ident32[:].rearrange("p (q c) -> p q c", q=1).to_broadcast(
                    [P, NPAIR, P]
                ),
                v,
            )
            iw.append(t)
        i7w, i15w, i13w = iw
        bias_t = pers.tile([P, NPAIR], f32, tag="bias")
        nc.sync.dma_start(bias_t[:], bias.rearrange("(f p) -> p f", p=P))
        ones2b = pers.tile([P, 2], bf16, tag="ones2b")
        nc.vector.memset(ones2b[:], 0.0)
        nc.vector.memset(ones2b[0:64, 0:1], 1.0)
        nc.vector.memset(ones2b[64:128, 1:2], 1.0)
        onescol = pers.tile([P, 1], bf16, tag="onescol")
        nc.vector.memset(onescol[:], 1.0)
        qsum4 = pers.tile([P, NPAIR, 64], f32, tag="qsum4")
        qkvt = big.tile([P, NPAIR, N_TOK], bf16, tag="qkvt")
        st = big.tile([P, NPAIR, N_TOK], bf16, tag="st")

        # ---------------- phase 1: projection (fp8 DoubleRow) ----------------
        with (
            tc.tile_pool(name="wq", bufs=1) as wqp,
            tc.tile_pool(name="x", bufs=3) as xpool,
            tc.tile_pool(name="pp", bufs=8, space="PSUM") as pp,
        ):
            whire = Whi.rearrange("(eo p) hd -> p eo hd", p=P).bitcast(fp8)
            wlore = Wlo.rearrange("(eo p) hd -> p eo hd", p=P).bitcast(fp8)
            xhire = Xhi.rearrange("(eo p) n -> p eo n", p=P).bitcast(fp8)
            xlore = Xlo.rearrange("(eo p) n -> p eo n", p=P).bitcast(fp8)

            xtiles = {}

            def load_x_half(c, h, xt=None):
                csl = slice(c * XCH, (c + 1) * XCH)
                if xt is None:
                    xt = xpool.tile([P, ECH, 2, XCH], fp8, tag="xt")
                nc.sync.dma_start(
                    xt[:, h * 4:(h + 1) * 4, 0, :],
                    xhire[:, h * 4:(h + 1) * 4, csl],
                )
                nc.sync.dma_start(
                    xt[:, h * 4:(h + 1) * 4, 1, :],
                    xlore[:, h * 4:(h + 1) * 4, csl],
                )
                xtiles[c] = xt
                return xt

            def load_x(c):
                xt = load_x_half(c, 0)
                load_x_half(c, 1, xt)

            # wdup holds Whi twice per eo (the DoubleRow hi/lo chain uses
            # the same stationary for both k-slots); wlo pairs eo chunks.
            # queue order matches first-use order: x0/wq for eo0-3 first.
            wdups = []

            def load_wdup(h):
                wd = wqp.tile([P, ECH // 2, 2, 512], fp8, tag=f"wdup{h}")
                for i in range(2):
                    nc.sync.dma_start(
                        wd[:, :, i, :], whire[:, h * 4:(h + 1) * 4, :]
                    )
                wdups.append(wd)

            # chunk 0 and wdup0 at finest granularity: the first matmul
            # only needs eo0-1 slices (512KB) to start
            xt0 = xpool.tile([P, ECH, 2, XCH], fp8, tag="xt")
            xtiles[0] = xt0
            wd0 = wqp.tile([P, ECH // 2, 2, 512], fp8, tag="wdup0")
            wdups.append(wd0)
            for q in range(2):
                qs = slice(q * 2, (q + 1) * 2)
                nc.sync.dma_start(xt0[:, qs, 0, :], xhire[:, qs, 0:XCH])
                nc.sync.dma_start(xt0[:, qs, 1, :], xlore[:, qs, 0:XCH])
                for i in range(2):
                    nc.sync.dma_start(wd0[:, qs, i, :], whire[:, qs, :])
            load_x_half(0, 1, xt0)
            load_wdup(1)
            wlo = wqp.tile([P, ECH, 512], fp8, tag="wlo")
            nc.sync.dma_start(wlo[:], wlore[:])
            load_x(1)

            for c in range(NCHP):
                csl = slice(c * XCH, (c + 1) * XCH)
                xt = xtiles.pop(c)
                for p in range(NPAIR):
                    psl = slice(p * P, (p + 1) * P)
                    ps = pp.tile([P, XCH], f32, tag="proj")
                    for e in range(ECH):
                        nc.tensor.matmul(
                            ps[:], wdups[e // 4][:, e % 4, :, psl], xt[:, e, :, :],
                            perf_mode=mybir.MatmulPerfMode.DoubleRow,
                            start=(e == 0), stop=False,
                            skip_group_check=True,
                        )
                    for h in range(ECH // 2):
                        nc.tensor.matmul(
                            ps[:], wlo[:, 2 * h:2 * h + 2, psl],
                            xt[:, 2 * h:2 * h + 2, 0, :],
                            perf_mode=mybir.MatmulPerfMode.DoubleRow,
                            start=False, stop=(h == ECH // 2 - 1),
                            skip_group_check=True,
                        )
                    # PSUM -> SBUF bf16 with 1/256 scale + bias fused
                    if p % 2 == 0:
                        nc.vector.tensor_scalar(
                            qkvt[:, p, csl], ps[:], 1.0 / 256,
                            bias_t[:, p:p + 1], ALU.mult, ALU.add,
                        )
                    else:
                        nc.scalar.activation(
                            qkvt[:, p, csl], ps[:], ACTF.Identity,
                            bias=bias_t[:, p:p + 1], scale=1.0 / 256,
                        )
                if c + 2 < NCHP:
                    load_x(c + 2)
                # landmark partial sums (post-bias): 8 windows/chunk/pair
                nw = XCH // 64
                nc.vector.tensor_reduce(
                    qsum4[:, :, c * nw:(c + 1) * nw],
                    qkvt[:, :, csl].rearrange("p q (w t) -> p q w t", t=64),
                    axis=AX.X, op=ALU.add,
                )

        # ---------------- phase 2+: everything else ----------------
        with (
            tc.tile_pool(name="wk", bufs=4) as wk,
            tc.tile_pool(name="nsps", bufs=2, space="PSUM") as nsps,
            tc.tile_pool(name="stps", bufs=2, space="PSUM") as stps,
            tc.tile_pool(name="trp", bufs=2, space="PSUM") as trp,
            tc.tile_pool(name="mps", bufs=1, space="PSUM") as mps,
            tc.tile_pool(name="cps", bufs=1, space="PSUM") as cps,
        ):
            # landmarks: blkq = qsum/64 (bias already included), block-diag
            blkq = pers.tile([P, NPAIR, P], bf16, tag="blkq")
            nc.vector.memset(blkq[:], 0.0)
            nc.vector.tensor_scalar_mul(
                blkq[0:64, :, 0:64], qsum4[0:64, :, :], 1.0 / 64
            )
            nc.vector.tensor_scalar_mul(
                blkq[64:128, :, 64:128], qsum4[64:128, :, :], 1.0 / 64
            )

            # Gamma -> GD (row-normalized), all 4 pairs in one bank
            psg = nsps.tile([P, NPAIR, P], f32, tag="nsb", name="psg")
            for p in range(NPAIR):
                nc.tensor.matmul(
                    psg[:, p, :], blkq[:, p, :], blkq[:, p, :],
                    start=(p == 0), stop=(p == NPAIR - 1),
                    skip_group_check=True,
                )
            g_all = wk.tile([P, NPAIR, P], f32, tag="g")
            nc.scalar.activation(g_all[:], psg[:], ACTF.Exp, scale=0.125)
            gs4 = wk.tile([P, NPAIR], f32, tag="gs4")
            nc.vector.tensor_reduce(
                gs4[0:64, :], g_all[0:64, :, 0:64], axis=AX.X, op=ALU.add
            )
            nc.vector.tensor_reduce(
                gs4[64:128, :], g_all[64:128, :, 64:128], axis=AX.X, op=ALU.add
            )
            gri = wk.tile([P, NPAIR], f32, tag="gri")
            nc.vector.reciprocal(gri[:], gs4[:])
            gdf = wk.tile([P, NPAIR, P], f32, tag="gdf")
            nc.vector.memset(gdf[:], 0.0)
            nc.vector.tensor_tensor(
                gdf[0:64, :, 0:64], g_all[0:64, :, 0:64],
                gri[0:64, :].rearrange("p (q o) -> p q o", o=1).to_broadcast(
                    [64, NPAIR, 64]
                ),
                ALU.mult,
            )
            nc.vector.tensor_tensor(
                gdf[64:128, :, 64:128], g_all[64:128, :, 64:128],
                gri[64:128, :].rearrange("p (q o) -> p q o", o=1).to_broadcast(
                    [64, NPAIR, 64]
                ),
                ALU.mult,
            )
            # compensated K: gd (hi) + gd_lo so NS inverts GD at ~f32 precision
            gd = pers.tile([P, NPAIR, P], bf16, tag="gd")
            nc.vector.tensor_copy(gd[:], gdf[:])
            gd_lo = pers.tile([P, NPAIR, P], bf16, tag="gd_lo")
            nc.vector.scalar_tensor_tensor(
                gd_lo[:], gd[:], -1.0, gdf[:], ALU.mult, ALU.add
            )

            # ---- fused: ST exp + token loop + NS iterations ----
            mbank = mps.tile([P, NPAIR, P], f32, tag="mbank")
            # one bank holds all 32 chunks' per-token S rowsums (rall);
            # the Delta rowsums come from the exps' accum_out (cparts)
            combo = cps.tile([P, TCH * NPAIR * 2], f32, tag="rall")
            rall = combo[:].rearrange("p (c q h) -> p c q h", q=NPAIR, h=2)
            cparts = pers.tile([P, NPAIR, NCH8], f32, tag="cparts")

            def emit_st_pair(j, p):
                jsl = slice(j * 512, (j + 1) * 512)
                psst = stps.tile([P, 512], f32, tag="stb", name=f"psst{j}_{p}")
                nc.tensor.matmul(
                    psst[:], blkq[:, p, :], qkvt[:, p, jsl],
                    start=True, stop=True,
                )
                nc.scalar.activation(
                    st[:, p, jsl], psst[:], ACTF.Exp, scale=0.125,
                    accum_out=cparts[:, p, j:j + 1],
                )

            def emit_token_chunk(c):
                tsl = slice(c * P, (c + 1) * P)
                tr = trp.tile([P, 2, NPAIR, P], bf16, tag="tr", name=f"tr{c}")
                for i, src in enumerate((qkvt, st)):
                    for p in range(NPAIR):
                        nc.tensor.matmul(
                            tr[:, i, p, :], src[:, p, tsl], identb[:],
                            is_transpose=True,
                            start=(i == 0 and p == 0),
                            stop=(i == 1 and p == NPAIR - 1),
                            skip_group_check=True,
                        )
                trn = wk.tile([P, 2, NPAIR, P], bf16, tag="trn", name=f"trn{c}")
                nc.vector.tensor_copy(trn[:], tr[:])
                qnb = trn[:, 0]
                snb = trn[:, 1]
                for p in range(NPAIR):
                    nc.tensor.matmul(
                        mbank[:, p, :], snb[:, p, :], qnb[:, p, :],
                        start=(c == 0 and p == 0),
                        stop=(c == TCH - 1 and p == NPAIR - 1),
                        skip_group_check=True,
                    )
                    nc.tensor.matmul(
                        rall[:, c, p, :], st[:, p, tsl], ones2b[:],
                        start=(c == 0 and p == 0),
                        stop=(c == TCH - 1 and p == NPAIR - 1),
                        skip_group_check=True,
                    )

            def ns_mm(out, mk, it_name):
                """out[:, p, :] = sum over (hi, lo) accumulated matmuls."""
                ops = []
                for p in range(NPAIR):
                    ops.append(mk(p))
                n = 0
                for p, pairs in enumerate(ops):
                    for lhsT, rhs in pairs:
                        n += 1
                        nc.tensor.matmul(
                            out[:, p, :], lhsT, rhs,
                            start=(n == 1),
                            stop=(n == sum(len(o) for o in ops)),
                            skip_group_check=True,
                        )

            def emit_ns_part1(it, v_in, vt_in):
                pskv = nsps.tile([P, NPAIR, P], f32, tag="nsb", name=f"pskv{it}")
                ns_mm(pskv, lambda p: [
                    (ktr[:, p, :], v_in[:, p, :]),
                    (ktr_lo[:, p, :], v_in[:, p, :]),
                ], it)
                pskvt = nsps.tile([P, NPAIR, P], f32, tag="nsb", name=f"pskvt{it}")
                ns_mm(pskvt, lambda p: [
                    (v_in[:, p, :], ktr[:, p, :]),
                    (v_in[:, p, :], ktr_lo[:, p, :]),
                ], it)
                kvt = nsp.tile([P, NPAIR, P], bf16, tag="kvt", name=f"kvt{it}")
                nc.scalar.copy(kvt[:], pskvt[:])
                a1 = nsp.tile([P, NPAIR, P], bf16, tag="a1", name=f"a1_{it}")
                nc.vector.scalar_tensor_tensor(
                    a1[:], pskv[:], -1.0, i7w[:], ALU.mult, ALU.add
                )
                psa2 = nsps.tile([P, NPAIR, P], f32, tag="nsb", name=f"psa2{it}")
                ns_mm(psa2, lambda p: [(kvt[:, p, :], a1[:, p, :])], it)
                a3 = nsp.tile([P, NPAIR, P], bf16, tag="a3", name=f"a3_{it}")
                nc.vector.scalar_tensor_tensor(
                    a3[:], psa2[:], -1.0, i15w[:], ALU.mult, ALU.add
                )
                return kvt, a3

            def emit_ns_part2(it, v_in, vt_in, kvt, a3):
                psa4 = nsps.tile([P, NPAIR, P], f32, tag="nsb", name=f"psa4{it}")
                ns_mm(psa4, lambda p: [(kvt[:, p, :], a3[:, p, :])], it)
                a5 = nsp.tile([P, NPAIR, P], bf16, tag="a5", name=f"a5_{it}")
                nc.vector.scalar_tensor_tensor(
                    a5[:], psa4[:], -1.0, i13w[:], ALU.mult, ALU.add
                )
                if it < NS_ITERS - 1:
                    psv = nsps.tile([P, NPAIR, P], f32, tag="nsb", name=f"psv{it}")
                    ns_mm(psv, lambda p: [(vt_in[:, p, :], a5[:, p, :])], it)
                    vn = nsp.tile([P, NPAIR, P], bf16, tag="v", name=f"v{it + 1}")
                    nc.scalar.mul(vn[:], psv[:], 0.25)
                else:
                    vn = v_in
                psvt2 = nsps.tile([P, NPAIR, P], f32, tag="nsb", name=f"psvt2{it}")
                ns_mm(psvt2, lambda p: [(a5[:, p, :], vt_in[:, p, :])], it)
                vtn = nsp.tile([P, NPAIR, P], bf16, tag="vt", name=f"vt{it + 1}")
                nc.scalar.mul(vtn[:], psvt2[:], 0.25)
                return vn, vtn

            # early ST block 0 + first two token chunks overlap the
            # serial NS-scale / NS-init chain below
            for p in range(NPAIR):
                emit_st_pair(0, p)
            emit_token_chunk(0)
            emit_token_chunk(1)

            # NS scale (per-core): rowsums of GD are exactly 1, so
            # scale = 1 / max colsum over the core's 8 heads.
            psc = nsps.tile([P, NPAIR, P], f32, tag="nsb", name="psc")
            for p in range(NPAIR):
                nc.tensor.matmul(
                    psc[0:2, p, :], ones2b[:], gd[:, p, :],
                    start=(p == 0), stop=(p == NPAIR - 1),
                    skip_group_check=True,
                )
            cm2 = wk.tile([2, 1], f32, tag="cm2")
            nc.vector.reduce_max(
                cm2[:], psc[0:2].rearrange("p q c -> p (q c)"), axis=AX.X
            )
            csep = wk.tile([P, 1], f32, tag="csep")
            nc.vector.memset(csep[:], -1e30)
            nc.vector.tensor_copy(csep[0:2, :], cm2[:])
            cmax = wk.tile([P, 1], f32, tag="cmax")
            nc.gpsimd.partition_all_reduce(
                cmax[:], csep[:], P, bass_isa.ReduceOp.max
            )
            sv = pers.tile([P, 1], f32, tag="sv")
            nc.vector.reciprocal(sv[:], cmax[:])

            # NS init: V0 = s*GD^T, V0^T = s*GD, K^T = GD^T (+lo residual)
            pskt = trp.tile([P, 2, NPAIR, P], bf16, tag="tr", name="pskt")
            for i, src in enumerate((gd, gd_lo)):
                for p in range(NPAIR):
                    nc.tensor.matmul(
                        pskt[:, i, p, :], src[:, p, :], identb[:],
                        is_transpose=True,
                        start=(i == 0 and p == 0),
                        stop=(i == 1 and p == NPAIR - 1),
                        skip_group_check=True,
                    )
            ktr2 = pers.tile([P, 2, NPAIR, P], bf16, tag="ktr2")
            nc.vector.tensor_copy(ktr2[:], pskt[:])
            ktr = ktr2[:, 0]
            ktr_lo = ktr2[:, 1]
            # init must span row(K^T) of the COMPENSATED K: include the
            # lo residual (the NS iteration never grows V's row space)
            v_cur = nsp.tile([P, NPAIR, P], bf16, tag="v", name="v0")
            nc.vector.scalar_tensor_tensor(
                v_cur[:], ktr2[:, 0], sv[:], ktr2[:, 1], ALU.mult, ALU.add
            )
            vt_cur = nsp.tile([P, NPAIR, P], bf16, tag="vt", name="vt0")
            nc.vector.scalar_tensor_tensor(
                vt_cur[:], gd[:], sv[:], gd_lo[:], ALU.mult, ALU.add
            )

            ns_state = None
            for j in range(NCH8):
                for i, c in enumerate(range(4 * j, 4 * j + 4)):
                    if c >= 2:
                        emit_token_chunk(c)
                    if j + 1 < NCH8:
                        emit_st_pair(j + 1, i)
                    if i == 1 and j < NS_ITERS:
                        ns_state = emit_ns_part1(j, v_cur, vt_cur)
                    elif i == 3 and j < NS_ITERS:
                        v_cur, vt_cur = emit_ns_part2(
                            j, v_cur, vt_cur, *ns_state
                        )

            # ---- precompute 1/r for every token (one wide reciprocal) ----
            rinv_all = pers.tile([P, TCH, NPAIR, 2], f32, tag="rinv_all")
            nc.vector.reciprocal(
                rinv_all[:].rearrange("p c q h -> p (c q h)"), combo[:]
            )
            nc.sync.dma_start(rinv_d.ap(), rinv_all[:])

            # ---- W = V6 @ diag(1/c) M ----
            csum = wk.tile([P, NPAIR], f32, tag="csum")
            nc.vector.tensor_reduce(
                csum[:], cparts[:], axis=AX.X, op=ALU.add
            )
            cinv = wk.tile([P, NPAIR], f32, tag="cinv")
            nc.vector.reciprocal(cinv[:], csum[:])
            dvp = wk.tile([P, NPAIR, P], bf16, tag="dvp")
            nc.vector.memset(dvp[:], 0.0)
            nc.vector.tensor_tensor(
                dvp[0:64, :, 0:64], mbank[0:64, :, 0:64],
                cinv[0:64, :].rearrange("p (q o) -> p q o", o=1).to_broadcast(
                    [64, NPAIR, 64]
                ),
                ALU.mult,
            )
            nc.vector.tensor_tensor(
                dvp[64:128, :, 64:128], mbank[64:128, :, 64:128],
                cinv[64:128, :].rearrange("p (q o) -> p q o", o=1).to_broadcast(
                    [64, NPAIR, 64]
                ),
                ALU.mult,
            )
            psw = nsps.tile([P, NPAIR, P], f32, tag="nsb", name="psw")
            for p in range(NPAIR):
                nc.tensor.matmul(
                    psw[:, p, :], vt_cur[:, p, :], dvp[:, p, :],
                    start=(p == 0), stop=(p == NPAIR - 1),
                    skip_group_check=True,
                )
            wpad = pers.tile([P, NPAIR, P], bf16, tag="wpad")
            nc.vector.tensor_copy(wpad[:], psw[:])

        # ---------------- final: out = diag(1/r) S W ----------------
        with (
            tc.tile_pool(name="fin", bufs=3) as fin,
            tc.tile_pool(name="pso", bufs=6, space="PSUM") as psop,
        ):
            ore = out_d.rearrange("(g i p) c -> g p i c", i=4, p=P)
            ot4 = None
            for c in range(TCH):
                tsl = slice(c * P, (c + 1) * P)
                g, i = divmod(c, 4)
                pso = psop.tile([P, NPAIR, P], f32, tag="pso", name=f"pso{c}")
                for p in range(NPAIR):
                    nc.tensor.matmul(
                        pso[:, p, :], st[:, p, tsl], wpad[:, p, :],
                        start=(p == 0), stop=(p == NPAIR - 1),
                        skip_group_check=True,
                    )
                if i == 0:
                    ot4 = fin.tile([P, 4, NPAIR, P], bf16, tag="ot", name=f"ot{g}")
                if c % 2 == 0:
                    nc.vector.tensor_copy(ot4[:, i], pso[:])
                else:
                    nc.scalar.copy(ot4[:, i], pso[:])
                if i == 3:
                    nc.sync.dma_start(
                        ore[g], ot4[:].rearrange("p i q c -> p i (q c)")
                    )

    nc.compile()
    return nc


def _get_nc():
    if "nc" not in _CACHE:
        _CACHE["nc"] = _build()
    return _CACHE["nc"]


def kernel(X, Wq, bq):
    from concourse.bass_utils import run_bass_kernel_spmd

    import ml_dtypes

    f8 = ml_dtypes.float8_e4m3fn
    nc = _get_nc()
    B, E, n = X.shape
    H = Wq.shape[0]

    def split8(a):
        hi = a.astype(f8)
        lo = (a - hi.astype(np.float32)).astype(f8)
        return hi.view(np.uint8), lo.view(np.uint8)

    xhl = [split8(8.0 * X[b]) for b in range(B)]
    in_maps = []
    for core in range(8):
        b = core // 2
        h0 = 8 * (core % 2)
        wq_c = Wq[h0:h0 + 8]                      # [8, 64, 1024]
        wqt_c = np.ascontiguousarray(wq_c.transpose(2, 0, 1).reshape(E, 512))
        whi, wlo = split8(32.0 * wqt_c)
        bias_c = np.ascontiguousarray(bq[h0:h0 + 8].reshape(512))
        in_maps.append({
            "Xhi": np.ascontiguousarray(xhl[b][0]),
            "Xlo": np.ascontiguousarray(xhl[b][1]),
            "Whi": np.ascontiguousarray(whi),
            "Wlo": np.ascontiguousarray(wlo),
            "bias": bias_c,
        })
    res = run_bass_kernel_spmd(nc, in_maps, core_ids=list(range(8)))
    out = np.empty((B, H, n, 64), dtype=np.float32)
    for core in range(8):
        b = core // 2
        h0 = 8 * (core % 2)
        oc = res.results[core]["out"].astype(np.float32).reshape(32, P, 8, 64)
        rv = res.results[core]["rinv"].transpose(1, 0, 2, 3).reshape(32, P, 8, 1)
        oc = (oc * rv).reshape(n, 8, 64)
        out[b, h0:h0 + 8] = oc.transpose(1, 0, 2)
    return out


# revision 44
# speedup vs baseline: 1.0025x; 1.0025x over previous
"""CoNystromAttention Trainium2 kernel.

Shard: 8 cores = 4 batches x 2 head-groups (8 heads each). Per core:
one batch b, 8 heads organized as 4 "pairs" (2 heads = 128 partitions).

Math (reference, with Q=K=V=QKV):
  QKV = X[b].T @ Wq[h].T + bq[h]                       [n=4096, d=64]
  Qt  = window-mean(QKV, 64)                           [m=64, d]
  S   = exp(QKV @ Qt.T / 8)     (Beta; Delta = S.T)    [n, m]
  G   = exp(Qt @ Qt.T / 8)
  GD  = G / rowsum(G);  V6 = newton_schulz(GD, 6)      (pinv)
  out = diag(1/r) S V6 diag(1/c) S.T QKV,  r=rowsum(S), c=colsum(S)

Projection via fp8e4 DoubleRow matmuls on host-prepped hi/lo splits of
8*X and 32*W (QKV = (Whi^T(Xhi+Xlo) + Wlo^T Xhi)/256, ~0.1% accurate);
everything downstream (S, transposes, M, NS, final) in bf16; output
DMA'd as bf16 and upcast on the host.  The NS operand K is
error-compensated as gd_hi + gd_lo (two accumulating bf16 matmuls) so
the iteration inverts GD at ~f32 precision.  NS init scale uses the
per-core max (8 heads) instead of the reference's global max; since GD
is row-normalized the rowsum term is exactly 1, so scale=1/max colsum.
"""

import numpy as np

P = 128
N_TOK = 4096
EMBED = 1024
NPAIR = 4            # head-pairs per core (8 heads)
ECH = EMBED // P     # 8 contraction chunks
XCH = 512            # projection chunk (tokens)
NCHP = N_TOK // XCH  # 8 projection chunks
NCH8 = N_TOK // 512  # 8 ST chunks of 512
TCH = N_TOK // P     # 32 token chunks of 128
NS_ITERS = 6

_CACHE = {}


def _build(**_ignored):
    import concourse.mybir as mybir
    from concourse import bacc, bass_isa
    from concourse.tile import TileContext
    from concourse.masks import make_identity

    f32 = mybir.dt.float32
    f32r = mybir.dt.float32r
    bf16 = mybir.dt.bfloat16
    ALU = mybir.AluOpType
    ACTF = mybir.ActivationFunctionType
    AX = mybir.AxisListType

    u8 = mybir.dt.uint8
    bf16_out = mybir.dt.bfloat16
    fp8 = mybir.dt.float8e4

    nc = bacc.Bacc("TRN2", target_bir_lowering=False, debug=False)
    # projection inputs as e4m3 hi/lo splits of 8*X and 32*W (host-prepped);
    # QKV = (Whi^T(Xhi+Xlo) + Wlo^T Xhi)/256 via DoubleRow fp8 matmuls
    Xhi = nc.dram_tensor("Xhi", [EMBED, N_TOK], u8, kind="ExternalInput")
    Xlo = nc.dram_tensor("Xlo", [EMBED, N_TOK], u8, kind="ExternalInput")
    Whi = nc.dram_tensor("Whi", [EMBED, 512], u8, kind="ExternalInput")
    Wlo = nc.dram_tensor("Wlo", [EMBED, 512], u8, kind="ExternalInput")
    bias = nc.dram_tensor("bias", [512], f32, kind="ExternalInput")
    out_d = nc.dram_tensor("out", [N_TOK, 512], bf16_out, kind="ExternalOutput")
    rinv_d = nc.dram_tensor("rinv", [P, TCH, NPAIR, 2], f32, kind="ExternalOutput")

    with TileContext(nc) as tc, (
        tc.tile_pool(name="big", bufs=1)
    ) as big, tc.tile_pool(name="pers", bufs=1) as pers, tc.tile_pool(
        name="nsv", bufs=2
    ) as nsp:
        # ---------------- persistent tiles ----------------
        ident32 = pers.tile([P, P], f32, tag="ident32")
        make_identity(nc, ident32[:])
        identb = pers.tile([P, P], bf16, tag="identb")
        nc.vector.tensor_copy(identb[:], ident32[:])
        iw = []
        for v in (7.0, 15.0, 13.0):
            t = pers.tile([P, NPAIR, P], f32, tag=f"i{int(v)}")
            nc.vector.tensor_scalar_mul(
                t[:],
                ident32[:].rearrange("p (q c) -> p q c", q=1).to_broadcast(
                    [P, NPAIR, P]
                ),
                v,
            )
            iw.append(t)
        i7w, i15w, i13w = iw
        bias_t = pers.tile([P, NPAIR], f32, tag="bias")
        nc.sync.dma_start(bias_t[:], bias.rearrange("(f p) -> p f", p=P))
        ones2b = pers.tile([P, 2], bf16, tag="ones2b")
        nc.vector.memset(ones2b[:], 0.0)
        nc.vector.memset(ones2b[0:64, 0:1], 1.0)
        nc.vector.memset(ones2b[64:128, 1:2], 1.0)
        onescol = pers.tile([P, 1], bf16, tag="onescol")
        nc.vector.memset(onescol[:], 1.0)
        qsum4 = pers.tile([P, NPAIR, 64], f32, tag="qsum4")
        qkvt = big.tile([P, NPAIR, N_TOK], bf16, tag="qkvt")
        st = big.tile([P, NPAIR, N_TOK], bf16, tag="st")

        # ---------------- phase 1: projection (fp8 DoubleRow) ----------------
        with (
            tc.tile_pool(name="wq", bufs=1) as wqp,
            tc.tile_pool(name="x", bufs=3) as xpool,
            tc.tile_pool(name="pp", bufs=8, space="PSUM") as pp,
        ):
            whire = Whi.rearrange("(eo p) hd -> p eo hd", p=P).bitcast(fp8)
            wlore = Wlo.rearrange("(eo p) hd -> p eo hd", p=P).bitcast(fp8)
            xhire = Xhi.rearrange("(eo p) n -> p eo n", p=P).bitcast(fp8)
            xlore = Xlo.rearrange("(eo p) n -> p eo n", p=P).bitcast(fp8)

            xtiles = {}

            def load_x_half(c, h, xt=None):
                csl = slice(c * XCH, (c + 1) * XCH)
                if xt is None:
                    xt = xpool.tile([P, ECH, 2, XCH], fp8, tag="xt")
                nc.sync.dma_start(
                    xt[:, h * 4:(h + 1) * 4, 0, :],
                    xhire[:, h * 4:(h + 1) * 4, csl],
                )
                nc.sync.dma_start(
                    xt[:, h * 4:(h + 1) * 4, 1, :],
                    xlore[:, h * 4:(h + 1) * 4, csl],
                )
                xtiles[c] = xt
                return xt

            def load_x(c):
                xt = load_x_half(c, 0)
                load_x_half(c, 1, xt)

            # wdup holds Whi twice per eo (the DoubleRow hi/lo chain uses
            # the same stationary for both k-slots); wlo pairs eo chunks.
            # queue order matches first-use order: x0/wq for eo0-3 first.
            wdups = []

            def load_wdup(h):
                wd = wqp.tile([P, ECH // 2, 2, 512], fp8, tag=f"wdup{h}")
                for i in range(2):
                    nc.sync.dma_start(
                        wd[:, :, i, :], whire[:, h * 4:(h + 1) * 4, :]
                    )
                wdups.append(wd)

            # chunk 0 and wdup0 at finest granularity: the first matmul
            # only needs eo0-1 slices (512KB) to start
            xt0 = xpool.tile([P, ECH, 2, XCH], fp8, tag="xt")
            xtiles[0] = xt0
            wd0 = wqp.tile([P, ECH // 2, 2, 512], fp8, tag="wdup0")
            wdups.append(wd0)
            for q in range(2):
                qs = slice(q * 2, (q + 1) * 2)
                nc.sync.dma_start(xt0[:, qs, 0, :], xhire[:, qs, 0:XCH])
                nc.sync.dma_start(xt0[:, qs, 1, :], xlore[:, qs, 0:XCH])
                for i in range(2):
                    nc.sync.dma_start(wd0[:, qs, i, :], whire[:, qs, :])
            load_x_half(0, 1, xt0)
            load_wdup(1)
            wlo = wqp.tile([P, ECH, 512], fp8, tag="wlo")
            nc.sync.dma_start(wlo[:], wlore[:])
            load_x(1)

            for c in range(NCHP):
                csl = slice(c * XCH, (c + 1) * XCH)
                xt = xtiles.pop(c)
                for p in range(NPAIR):
                    psl = slice(p * P, (p + 1) * P)
                    ps = pp.tile([P, XCH], f32, tag="proj")
                    for e in range(ECH):
                        nc.tensor.matmul(
                            ps[:], wdups[e // 4][:, e % 4, :, psl], xt[:, e, :, :],
                            perf_mode=mybir.MatmulPerfMode.DoubleRow,
                            start=(e == 0), stop=False,
                            skip_group_check=True,
                        )
                    for h in range(ECH // 2):
                        nc.tensor.matmul(
                            ps[:], wlo[:, 2 * h:2 * h + 2, psl],
                            xt[:, 2 * h:2 * h + 2, 0, :],
                            perf_mode=mybir.MatmulPerfMode.DoubleRow,
                            start=False, stop=(h == ECH // 2 - 1),
                            skip_group_check=True,
                        )
                    # PSUM -> SBUF bf16 with 1/256 scale + bias fused
                    if p % 2 == 0:
                        nc.vector.tensor_scalar(
                            qkvt[:, p, csl], ps[:], 1.0 / 256,
                            bias_t[:, p:p + 1], ALU.mult, ALU.add,
                        )
                    else:
                        nc.scalar.activation(
                            qkvt[:, p, csl], ps[:], ACTF.Identity,
                            bias=bias_t[:, p:p + 1], scale=1.0 / 256,
                        )
                if c + 2 < NCHP:
                    load_x(c + 2)
                # landmark partial sums (post-bias): 8 windows/chunk/pair
                nw = XCH // 64
                nc.vector.tensor_reduce(
                    qsum4[:, :, c * nw:(c + 1) * nw],
                    qkvt[:, :, csl].rearrange("p q (w t) -> p q w t", t=64),
                    axis=AX.X, op=ALU.add,
                )

        # ---------------- phase 2+: everything else ----------------
        with (
            tc.tile_pool(name="wk", bufs=4) as wk,
            tc.tile_pool(name="nsps", bufs=2, space="PSUM") as nsps,
            tc.tile_pool(name="stps", bufs=2, space="PSUM") as stps,
            tc.tile_pool(name="trp", bufs=2, space="PSUM") as trp,
            tc.tile_pool(name="mps", bufs=1, space="PSUM") as mps,
            tc.tile_pool(name="cps", bufs=1, space="PSUM") as cps,
        ):
            # landmarks: blkq = qsum/64 (bias already included), block-diag
            blkq = pers.tile([P, NPAIR, P], bf16, tag="blkq")
            nc.vector.memset(blkq[:], 0.0)
            nc.vector.tensor_scalar_mul(
                blkq[0:64, :, 0:64], qsum4[0:64, :, :], 1.0 / 64
            )
            nc.vector.tensor_scalar_mul(
                blkq[64:128, :, 64:128], qsum4[64:128, :, :], 1.0 / 64
            )

            # Gamma -> GD (row-normalized), all 4 pairs in one bank
            psg = nsps.tile([P, NPAIR, P], f32, tag="nsb", name="psg")
            for p in range(NPAIR):
                nc.tensor.matmul(
                    psg[:, p, :], blkq[:, p, :], blkq[:, p, :],
                    start=(p == 0), stop=(p == NPAIR - 1),
                    skip_group_check=True,
                )
            g_all = wk.tile([P, NPAIR, P], f32, tag="g")
            nc.scalar.activation(g_all[:], psg[:], ACTF.Exp, scale=0.125)
            gs4 = wk.tile([P, NPAIR], f32, tag="gs4")
            nc.vector.tensor_reduce(
                gs4[0:64, :], g_all[0:64, :, 0:64], axis=AX.X, op=ALU.add
            )
            nc.vector.tensor_reduce(
                gs4[64:128, :], g_all[64:128, :, 64:128], axis=AX.X, op=ALU.add
            )
            gri = wk.tile([P, NPAIR], f32, tag="gri")
            nc.vector.reciprocal(gri[:], gs4[:])
            gdf = wk.tile([P, NPAIR, P], f32, tag="gdf")
            nc.vector.memset(gdf[:], 0.0)
            nc.vector.tensor_tensor(
                gdf[0:64, :, 0:64], g_all[0:64, :, 0:64],
                gri[0:64, :].rearrange("p (q o) -> p q o", o=1).to_broadcast(
                    [64, NPAIR, 64]
                ),
                ALU.mult,
            )
            nc.vector.tensor_tensor(
                gdf[64:128, :, 64:128], g_all[64:128, :, 64:128],
                gri[64:128, :].rearrange("p (q o) -> p q o", o=1).to_broadcast(
                    [64, NPAIR, 64]
                ),
                ALU.mult,
            )
            # compensated K: gd (hi) + gd_lo so NS inverts GD at ~f32 precision
            gd = pers.tile([P, NPAIR, P], bf16, tag="gd")
            nc.vector.tensor_copy(gd[:], gdf[:])
            gd_lo = pers.tile([P, NPAIR, P], bf16, tag="gd_lo")
            nc.vector.scalar_tensor_tensor(
                gd_lo[:], gd[:], -1.0, gdf[:], ALU.mult, ALU.add
            )

            # ---- fused: ST exp + token loop + NS iterations ----
            mbank = mps.tile([P, NPAIR, P], f32, tag="mbank")
            # one bank holds all 32 chunks' per-token S rowsums (rall);
            # the Delta rowsums come from the exps' accum_out (cparts)
            combo = cps.tile([P, TCH * NPAIR * 2], f32, tag="rall")
            rall = combo[:].rearrange("p (c q h) -> p c q h", q=NPAIR, h=2)
            cparts = pers.tile([P, NPAIR, NCH8], f32, tag="cparts")

            def emit_st_pair(j, p):
                jsl = slice(j * 512, (j + 1) * 512)
                psst = stps.tile([P, 512], f32, tag="stb", name=f"psst{j}_{p}")
                nc.tensor.matmul(
                    psst[:], blkq[:, p, :], qkvt[:, p, jsl],
                    start=True, stop=True,
                )
                nc.scalar.activation(
                    st[:, p, jsl], psst[:], ACTF.Exp, scale=0.125,
                    accum_out=cparts[:, p, j:j + 1],
                )

            def emit_token_chunk(c):
                tsl = slice(c * P, (c + 1) * P)
                tr = trp.tile([P, 2, NPAIR, P], bf16, tag="tr", name=f"tr{c}")
                for i, src in enumerate((qkvt, st)):
                    for p in range(NPAIR):
                        nc.tensor.matmul(
                            tr[:, i, p, :], src[:, p, tsl], identb[:],
                            is_transpose=True,
                            start=(i == 0 and p == 0),
                            stop=(i == 1 and p == NPAIR - 1),
                            skip_group_check=True,
                        )
                trn = wk.tile([P, 2, NPAIR, P], bf16, tag="trn", name=f"trn{c}")
                nc.vector.tensor_copy(trn[:], tr[:])
                qnb = trn[:, 0]
                snb = trn[:, 1]
                for p in range(NPAIR):
                    nc.tensor.matmul(
                        mbank[:, p, :], snb[:, p, :], qnb[:, p, :],
                        start=(c == 0 and p == 0),
                        stop=(c == TCH - 1 and p == NPAIR - 1),
                        skip_group_check=True,
                    )
                    nc.tensor.matmul(
                        rall[:, c, p, :], st[:, p, tsl], ones2b[:],
                        start=(c == 0 and p == 0),
                        stop=(c == TCH - 1 and p == NPAIR - 1),
                        skip_group_check=True,
                    )

            def ns_mm(out, mk, it_name):
                """out[:, p, :] = sum over (hi, lo) accumulated matmuls."""
                ops = []
                for p in range(NPAIR):
                    ops.append(mk(p))
                n = 0
                for p, pairs in enumerate(ops):
                    for lhsT, rhs in pairs:
                        n += 1
                        nc.tensor.matmul(
                            out[:, p, :], lhsT, rhs,
                            start=(n == 1),
                            stop=(n == sum(len(o) for o in ops)),
                            skip_group_check=True,
                        )

            def emit_ns_part1(it, v_in, vt_in):
                pskv = nsps.tile([P, NPAIR, P], f32, tag="nsb", name=f"pskv{it}")
                ns_mm(pskv, lambda p: [
                    (ktr[:, p, :], v_in[:, p, :]),
                    (ktr_lo[:, p, :], v_in[:, p, :]),
                ], it)
                pskvt = nsps.tile([P, NPAIR, P], f32, tag="nsb", name=f"pskvt{it}")
                ns_mm(pskvt, lambda p: [
                    (v_in[:, p, :], ktr[:, p, :]),
                    (v_in[:, p, :], ktr_lo[:, p, :]),
                ], it)
                kvt = nsp.tile([P, NPAIR, P], bf16, tag="kvt", name=f"kvt{it}")
                nc.scalar.copy(kvt[:], pskvt[:])
                a1 = nsp.tile([P, NPAIR, P], bf16, tag="a1", name=f"a1_{it}")
                nc.vector.scalar_tensor_tensor(
                    a1[:], pskv[:], -1.0, i7w[:], ALU.mult, ALU.add
                )
                psa2 = nsps.tile([P, NPAIR, P], f32, tag="nsb", name=f"psa2{it}")
                ns_mm(psa2, lambda p: [(kvt[:, p, :], a1[:, p, :])], it)
                a3 = nsp.tile([P, NPAIR, P], bf16, tag="a3", name=f"a3_{it}")
                nc.vector.scalar_tensor_tensor(
                    a3[:], psa2[:], -1.0, i15w[:], ALU.mult, ALU.add
                )
                return kvt, a3

            def emit_ns_part2(it, v_in, vt_in, kvt, a3):
                psa4 = nsps.tile([P, NPAIR, P], f32, tag="nsb", name=f"psa4{it}")
                ns_mm(psa4, lambda p: [(kvt[:, p, :], a3[:, p, :])], it)
                a5 = nsp.tile([P, NPAIR, P], bf16, tag="a5", name=f"a5_{it}")
                nc.vector.scalar_tensor_tensor(
                    a5[:], psa4[:], -1.0, i13w[:], ALU.mult, ALU.add
                )
                if it < NS_ITERS - 1:
                    psv = nsps.tile([P, NPAIR, P], f32, tag="nsb", name=f"psv{it}")
                    ns_mm(psv, lambda p: [(vt_in[:, p, :], a5[:, p, :])], it)
                    vn = nsp.tile([P, NPAIR, P], bf16, tag="v", name=f"v{it + 1}")
                    nc.vector.tensor_scalar_mul(vn[:], psv[:], 0.25)
                else:
                    vn = v_in
                psvt2 = nsps.tile([P, NPAIR, P], f32, tag="nsb", name=f"psvt2{it}")
                ns_mm(psvt2, lambda p: [(a5[:, p, :], vt_in[:, p, :])], it)
                vtn = nsp.tile([P, NPAIR, P], bf16, tag="vt", name=f"vt{it + 1}")
                nc.scalar.mul(vtn[:], psvt2[:], 0.25)
                return vn, vtn

            # early ST block 0 + first two token chunks overlap the
            # serial NS-scale / NS-init chain below
            for p in range(NPAIR):
                emit_st_pair(0, p)
            emit_token_chunk(0)
            emit_token_chunk(1)

            # NS scale (per-core): rowsums of GD are exactly 1, so
            # scale = 1 / max colsum over the core's 8 heads.
            psc = nsps.tile([P, NPAIR, P], f32, tag="nsb", name="psc")
            for p in range(NPAIR):
                nc.tensor.matmul(
                    psc[0:2, p, :], ones2b[:], gd[:, p, :],
                    start=(p == 0), stop=(p == NPAIR - 1),
                    skip_group_check=True,
                )
            cm2 = wk.tile([2, 1], f32, tag="cm2")
            nc.vector.reduce_max(
                cm2[:], psc[0:2].rearrange("p q c -> p (q c)"), axis=AX.X
            )
            csep = wk.tile([P, 1], f32, tag="csep")
            nc.vector.memset(csep[:], -1e30)
            nc.vector.tensor_copy(csep[0:2, :], cm2[:])
            cmax = wk.tile([P, 1], f32, tag="cmax")
            nc.gpsimd.partition_all_reduce(
                cmax[:], csep[:], P, bass_isa.ReduceOp.max
            )
            sv = pers.tile([P, 1], f32, tag="sv")
            nc.vector.reciprocal(sv[:], cmax[:])

            # NS init: V0 = s*GD^T, V0^T = s*GD, K^T = GD^T (+lo residual)
            pskt = trp.tile([P, 2, NPAIR, P], bf16, tag="tr", name="pskt")
            for i, src in enumerate((gd, gd_lo)):
                for p in range(NPAIR):
                    nc.tensor.matmul(
                        pskt[:, i, p, :], src[:, p, :], identb[:],
                        is_transpose=True,
                        start=(i == 0 and p == 0),
                        stop=(i == 1 and p == NPAIR - 1),
                        skip_group_check=True,
                    )
            ktr2 = pers.tile([P, 2, NPAIR, P], bf16, tag="ktr2")
            nc.vector.tensor_copy(ktr2[:], pskt[:])
            ktr = ktr2[:, 0]
            ktr_lo = ktr2[:, 1]
            # init must span row(K^T) of the COMPENSATED K: include the
            # lo residual (the NS iteration never grows V's row space)
            v_cur = nsp.tile([P, NPAIR, P], bf16, tag="v", name="v0")
            nc.vector.scalar_tensor_tensor(
                v_cur[:], ktr2[:, 0], sv[:], ktr2[:, 1], ALU.mult, ALU.add
            )
            vt_cur = nsp.tile([P, NPAIR, P], bf16, tag="vt", name="vt0")
            nc.vector.scalar_tensor_tensor(
                vt_cur[:], gd[:], sv[:], gd_lo[:], ALU.mult, ALU.add
            )

            ns_state = None
            for j in range(NCH8):
                for i, c in enumerate(range(4 * j, 4 * j + 4)):
                    if c >= 2:
                        emit_token_chunk(c)
                    if j + 1 < NCH8:
                        emit_st_pair(j + 1, i)
                    if i == 1 and j < NS_ITERS:
                        ns_state = emit_ns_part1(j, v_cur, vt_cur)
                    elif i == 3 and j < NS_ITERS:
                        v_cur, vt_cur = emit_ns_part2(
                            j, v_cur, vt_cur, *ns_state
                        )

            # ---- precompute 1/r for every token (one wide reciprocal) ----
            rinv_all = pers.tile([P, TCH, NPAIR, 2], f32, tag="rinv_all")
            nc.vector.reciprocal(
                rinv_all[:].rearrange("p c q h -> p (c q h)"), combo[:]
            )
            nc.sync.dma_start(rinv_d.ap(), rinv_all[:])

            # ---- W = V6 @ diag(1/c) M ----
            csum = wk.tile([P, NPAIR], f32, tag="csum")
            nc.vector.tensor_reduce(
                csum[:], cparts[:], axis=AX.X, op=ALU.add
            )
            cinv = wk.tile([P, NPAIR], f32, tag="cinv")
            nc.vector.reciprocal(cinv[:], csum[:])
            dvp = wk.tile([P, NPAIR, P], bf16, tag="dvp")
            nc.vector.memset(dvp[:], 0.0)
            nc.vector.tensor_tensor(
                dvp[0:64, :, 0:64], mbank[0:64, :, 0:64],
                cinv[0:64, :].rearrange("p (q o) -> p q o", o=1).to_broadcast(
                    [64, NPAIR, 64]
                ),
                ALU.mult,
            )
            nc.vector.tensor_tensor(
                dvp[64:128, :, 64:128], mbank[64:128, :, 64:128],
                cinv[64:128, :].rearrange("p (q o) -> p q o", o=1).to_broadcast(
                    [64, NPAIR, 64]
                ),
                ALU.mult,
            )
            psw = nsps.tile([P, NPAIR, P], f32, tag="nsb", name="psw")
            for p in range(NPAIR):
                nc.tensor.matmul(
                    psw[:, p, :], vt_cur[:, p, :], dvp[:, p, :],
                    start=(p == 0), stop=(p == NPAIR - 1),
                    skip_group_check=True,
                )
            wpad = pers.tile([P, NPAIR, P], bf16, tag="wpad")
            nc.vector.tensor_copy(wpad[:], psw[:])

        # ---------------- final: out = diag(1/r) S W ----------------
        with (
            tc.tile_pool(name="fin", bufs=3) as fin,
            tc.tile_pool(name="pso", bufs=6, space="PSUM") as psop,
        ):
            ore = out_d.rearrange("(g i p) c -> g p i c", i=4, p=P)
            ot4 = None
            for c in range(TCH):
                tsl = slice(c * P, (c + 1) * P)
                g, i = divmod(c, 4)
                pso = psop.tile([P, NPAIR, P], f32, tag="pso", name=f"pso{c}")
                for p in range(NPAIR):
                    nc.tensor.matmul(
                        pso[:, p, :], st[:, p, tsl], wpad[:, p, :],
                        start=(p == 0), stop=(p == NPAIR - 1),
                        skip_group_check=True,
                    )
                if i == 0:
                    ot4 = fin.tile([P, 4, NPAIR, P], bf16, tag="ot", name=f"ot{g}")
                if c % 2 == 0:
                    nc.vector.tensor_copy(ot4[:, i], pso[:])
                else:
                    nc.scalar.copy(ot4[:, i], pso[:])
                if i == 3:
                    nc.sync.dma_start(
                        ore[g], ot4[:].rearrange("p i q c -> p i (q c)")
                    )

    nc.compile()
    return nc


def _get_nc():
    if "nc" not in _CACHE:
        _CACHE["nc"] = _build()
    return _CACHE["nc"]


def kernel(X, Wq, bq):
    from concourse.bass_utils import run_bass_kernel_spmd

    import ml_dtypes

    f8 = ml_dtypes.float8_e4m3fn
    nc = _get_nc()
    B, E, n = X.shape
    H = Wq.shape[0]

    def split8(a):
        hi = a.astype(f8)
        lo = (a - hi.astype(np.float32)).astype(f8)
        return hi.view(np.uint8), lo.view(np.uint8)

    xhl = [split8(8.0 * X[b]) for b in range(B)]
    in_maps = []
    for core in range(8):
        b = core // 2
        h0 = 8 * (core % 2)
        wq_c = Wq[h0:h0 + 8]                      # [8, 64, 1024]
        wqt_c = np.ascontiguousarray(wq_c.transpose(2, 0, 1).reshape(E, 512))
        whi, wlo = split8(32.0 * wqt_c)
        bias_c = np.ascontiguousarray(bq[h0:h0 + 8].reshape(512))
        in_maps.append({
            "Xhi": np.ascontiguousarray(xhl[b][0]),
            "Xlo": np.ascontiguousarray(xhl[b][1]),
            "Whi": np.ascontiguousarray(whi),
            "Wlo": np.ascontiguousarray(wlo),
            "bias": bias_c,
        })
    res = run_bass_kernel_spmd(nc, in_maps, core_ids=list(range(8)))
    out = np.empty((B, H, n, 64), dtype=np.float32)
    for core in range(8):
        b = core // 2
        h0 = 8 * (core % 2)
        oc = res.results[core]["out"].astype(np.float32).reshape(32, P, 8, 64)
        rv = res.results[core]["rinv"].transpose(1, 0, 2, 3).reshape(32, P, 8, 1)
        oc = (oc * rv).reshape(n, 8, 64)
        out[b, h0:h0 + 8] = oc.transpose(1, 0, 2)
    return out
